# revision 6
# baseline (speedup 1.0000x reference)
"""CIN (xDeepFM CompressedInteractionNetwork) forward on 8 TRN2 NeuronCores.

v2: fp8 DoubleRow matmuls for L1/L2 (2 k-subtiles per PE instruction),
L0 exact in bf16 from host-precomputed symmetrized pair products.

Strategy (pure data parallelism, hardcoded from the problem spec):
  - batch 4096 -> 512 per core; 64 tiles of 8 batch elements; matmul free
    dim = 512 columns = (8 batch x 64 embed).
  - L0 exploits z0 = x (x) x symmetry: 1024 channels fold to 528 unordered
    pairs (symmetrized W0), padded to 768 = 6x128. The pair products are
    HOST-precomputed (input-only data) and DMA'd as bf16 -> L0 is exact,
    6 plain bf16 matmuls per out-half.
  - L1/L2 run in fp8(e4m3) DoubleRow: weights scaled x64 and split into
    hi + unscaled-lo residual (both fp8; lo lands in denormals and still
    recovers most of the quantization error -> rel err ~1e-2). hi+lo
    chains accumulate into one fp32 PSUM; ScalarE applies 1/64 + bias +
    relu on the way out.
  - z1/z2 (x (x) h products) built on DVE from bf16 xr (partition-
    broadcast DMA) and bf16 h: z1-half0 via bf16 build + ScalarE convert
    (every 8th tile: GpSimd convert), z1-half1/z2 via direct fp8-output
    tensor_tensor. This mixed scheme measures every engine's fp8 rate.
  - pipeline: PE iteration i = [L0(i), L2(i-2), L1(i)]; z2(i) is built at
    the end of iteration i and consumed two iterations later, so the PE
    never waits on the h2 -> z2 chain.
  - pooling (sum over embed) on DVE (r2b's reduce on GpSimd); final FC on
    host.
"""

import sys

sys.path.insert(0, "/opt/trn_rl_repo")

import numpy as np
import ml_dtypes
from contextlib import ExitStack

N_CORES = 8
B = 4096
F = 32
E = 64
BC = B // N_CORES  # 512 batch elements per core
NB = 8             # batch elements per tile
COLS = NB * E      # 512 matmul columns per tile
NT = BC // NB      # 64 tiles per core
O = 256            # conv out channels per layer
NP0 = 6            # L0 k-subtiles: 528 pairs padded to 768 = 6*128
S12 = 64.0         # fp8 weight scale for L1/L2

_CACHE = {}


def _build(n_tiles=NT):
    import concourse.bass as bass  # noqa: F401
    import concourse.mybir as mybir
    import concourse.tile as tile
    from concourse import bacc

    dt = mybir.dt
    AF = mybir.ActivationFunctionType
    ALU = mybir.AluOpType
    AX = mybir.AxisListType
    DR = mybir.MatmulPerfMode.DoubleRow

    nc = bacc.Bacc("TRN2", target_bir_lowering=False, debug=False,
                   num_devices=N_CORES)

    # host-precomputed symmetrized pair products for L0 (bf16, exact path)
    z0q = nc.declare_dram_parameter("z0q", [n_tiles, 128, NP0 * COLS],
                                    dt.bfloat16, isOutput=False)
    xtile = nc.declare_dram_parameter("xtile", [n_tiles, F, COLS],
                                      dt.bfloat16, isOutput=False)
    w0t = nc.declare_dram_parameter("w0t", [128, NP0 * O], dt.bfloat16,
                                    isOutput=False)
    # w1/w2: fp8 hi+lo residual pairs, layout [p, m, hl, g, o']
    w1t = nc.declare_dram_parameter("w1t", [128, 2 * 2 * 32 * 128],
                                    dt.float8e4, isOutput=False)
    w2t = nc.declare_dram_parameter("w2t", [128, 2 * 2 * 32 * 128],
                                    dt.float8e4, isOutput=False)
    b0 = nc.declare_dram_parameter("b0", [O], dt.float32, isOutput=False)
    b1 = nc.declare_dram_parameter("b1", [O], dt.float32, isOutput=False)
    b2 = nc.declare_dram_parameter("b2", [O], dt.float32, isOutput=False)
    pout = nc.declare_dram_parameter("pout", [4, 128, n_tiles * NB],
                                     dt.float32, isOutput=True)

    with ExitStack() as ctx:
        tc = ctx.enter_context(tile.TileContext(nc))
        const = ctx.enter_context(tc.tile_pool(name="const", bufs=1))

        lw0 = const.tile([128, NP0, O], dt.bfloat16)
        lw1 = const.tile([128, 2, 2, 32, 128], dt.float8e4)  # [p,m,hl,g,o']
        lw2 = const.tile([128, 2, 2, 32, 128], dt.float8e4)
        bias0 = const.tile([128, 2], dt.float32)
        bias1 = const.tile([128, 2], dt.float32)
        bias2 = const.tile([128, 2], dt.float32)

        # pooled accumulators [o_chunk 128, batch 512]
        P0 = const.tile([128, n_tiles * NB], dt.float32)
        P1 = const.tile([128, n_tiles * NB], dt.float32)
        P2a = const.tile([128, n_tiles * NB], dt.float32)
        P2b = const.tile([128, n_tiles * NB], dt.float32)

        # rotating pools
        z0_pool = ctx.enter_context(tc.tile_pool(name="z0", bufs=2))
        xr_pool = ctx.enter_context(tc.tile_pool(name="xr", bufs=2))
        z1b_pool = ctx.enter_context(tc.tile_pool(name="z1b", bufs=1))
        z1q_pool = ctx.enter_context(tc.tile_pool(name="z1q", bufs=1))
        z2q_pool = ctx.enter_context(tc.tile_pool(name="z2q", bufs=4))
        h_pool = ctx.enter_context(tc.tile_pool(name="h", bufs=2))
        r_pool = ctx.enter_context(tc.tile_pool(name="r", bufs=2))
        psum_pool = ctx.enter_context(tc.tile_pool(name="ps", bufs=8, space="PSUM"))

        z0t = [None] * n_tiles
        xrt = [None] * n_tiles
        z2t = [None] * n_tiles

        def emit_dma(t):
            z0 = z0_pool.tile([128, NP0, COLS], dt.bfloat16)
            nc.sync.dma_start(z0[:].rearrange("p g c -> p (g c)"), z0q.ap()[t])
            z0t[t] = z0
            xr = xr_pool.tile([128, F, COLS], dt.bfloat16)
            src = xtile.ap()[t].unsqueeze(0).broadcast_to([128, F, COLS])
            nc.sync.dma_start(xr[:], src)
            xrt[t] = xr

        def reduce_into(P, t, r_t, eng=None):
            (eng or nc.vector).tensor_reduce(
                P[:, t * NB:(t + 1) * NB],
                r_t[:].rearrange("p (b e) -> p b e", e=E), AX.X, ALU.add)

        # ---- preamble ----
        emit_dma(0)
        nc.sync.dma_start(lw0[:], w0t.ap().rearrange("p (g o) -> p g o", o=O))
        nc.sync.dma_start(bias0[:], b0.ap().rearrange("(m p) -> p m", p=128))
        w1v = w1t.ap().rearrange("p (m h x) -> p m h x", m=2, h=2)
        w2v = w2t.ap().rearrange("p (m h x) -> p m h x", m=2, h=2)
        for m in (1, 0):
            for hl in (0, 1):
                nc.sync.dma_start(
                    lw1[:, m, hl].rearrange("p g o -> p (g o)"), w1v[:, m, hl])
        nc.sync.dma_start(bias1[:], b1.ap().rearrange("(m p) -> p m", p=128))
        for m in (0, 1):
            for hl in (0, 1):
                nc.sync.dma_start(
                    lw2[:, m, hl].rearrange("p g o -> p (g o)"), w2v[:, m, hl])
        nc.sync.dma_start(bias2[:], b2.ap().rearrange("(m p) -> p m", p=128))

        for i in range(n_tiles + 2):
            if i + 1 < n_tiles:
                emit_dma(i + 1)

            if i < n_tiles:
                # -- PE: L0(i), bf16 exact; m=1 (h-half) first --
                ps0 = {m: psum_pool.tile([128, COLS], dt.float32,
                                         name=f"ps0{m}", tag="ps")
                       for m in (1, 0)}
                for m in (1, 0):
                    for g in range(NP0):
                        nc.tensor.matmul(
                            ps0[m][:], lw0[:, g, m * 128:(m + 1) * 128],
                            z0t[i][:, g, :], start=(g == 0), stop=(g == NP0 - 1))
                h1 = h_pool.tile([128, COLS], dt.bfloat16, name="h1", tag="h")
                nc.scalar.activation(h1[:], ps0[1][:], AF.Relu, bias=bias0[:, 1:2])
                r0 = r_pool.tile([128, COLS], dt.bfloat16, name="r0", tag="r")
                nc.scalar.activation(r0[:], ps0[0][:], AF.Relu, bias=bias0[:, 0:1])

                # -- DVE: z1 builds --
                z1b = z1b_pool.tile([128, 16, COLS], dt.bfloat16,
                                    name="z1b", tag="z1b")
                nc.vector.tensor_tensor(
                    z1b[:], xrt[i][:, 0:16, :],
                    h1[:].unsqueeze(1).broadcast_to([128, 16, COLS]), ALU.mult)
                z1q1 = z1q_pool.tile([128, 16, COLS], dt.float8e4,
                                     name="z1q1", tag="z1q1")
                nc.vector.tensor_tensor(
                    z1q1[:], xrt[i][:, 16:32, :],
                    h1[:].unsqueeze(1).broadcast_to([128, 16, COLS]), ALU.mult)
                # convert half0 bf16 -> fp8 (ScalarE; every 8th tile GpSimd)
                z1q0 = z1q_pool.tile([128, 16, COLS], dt.float8e4,
                                     name="z1q0", tag="z1q0")
                if i % 8 == 7:
                    nc.gpsimd.tensor_copy(z1q0[:], z1b[:])
                else:
                    nc.scalar.activation(z1q0[:], z1b[:], AF.Copy)
                z1h = {0: z1q0, 1: z1q1}

            if i >= 2:
                # -- PE: L2(i-2), fp8 DR hi+lo --
                c = i - 2
                ps2 = {m: psum_pool.tile([128, COLS], dt.float32,
                                         name=f"ps2{m}", tag="ps")
                       for m in (0, 1)}
                for m in (0, 1):
                    for hl in (0, 1):
                        for half in range(2):
                            for gp in range(8):
                                nc.tensor.matmul(
                                    ps2[m][:],
                                    lw2[:, m, hl,
                                        half * 16 + 2 * gp:half * 16 + 2 * gp + 2, :],
                                    z2t[c][half][:, 2 * gp:2 * gp + 2, :],
                                    start=(hl == 0 and half == 0 and gp == 0),
                                    stop=(hl == 1 and half == 1 and gp == 7),
                                    perf_mode=DR)
                r2a = r_pool.tile([128, COLS], dt.bfloat16, name="r2a", tag="r")
                nc.scalar.activation(r2a[:], ps2[0][:], AF.Relu,
                                     bias=bias2[:, 0:1], scale=1.0 / S12)
                r2b = r_pool.tile([128, COLS], dt.bfloat16, name="r2b", tag="r")
                nc.scalar.activation(r2b[:], ps2[1][:], AF.Relu,
                                     bias=bias2[:, 1:2], scale=1.0 / S12)

            # lagged reduces (kept off the critical path)
            if i < n_tiles:
                reduce_into(P0, i, r0)
            if i >= 2:
                reduce_into(P2a, i - 2, r2a)
                reduce_into(P2b, i - 2, r2b)

            if i < n_tiles:
                # -- PE: L1(i), fp8 DR hi+lo; m=1 first, half1 (direct) first --
                ps1 = {m: psum_pool.tile([128, COLS], dt.float32,
                                         name=f"ps1{m}", tag="ps")
                       for m in (1, 0)}
                for m in (1, 0):
                    first = True
                    for hl in (0, 1):
                        for half in (1, 0):
                            for gp in range(8):
                                nc.tensor.matmul(
                                    ps1[m][:],
                                    lw1[:, m, hl,
                                        half * 16 + 2 * gp:half * 16 + 2 * gp + 2, :],
                                    z1h[half][:, 2 * gp:2 * gp + 2, :],
                                    start=first,
                                    stop=(hl == 1 and half == 0 and gp == 7),
                                    perf_mode=DR)
                                first = False
                h2 = h_pool.tile([128, COLS], dt.bfloat16, name="h2", tag="h")
                nc.scalar.activation(h2[:], ps1[1][:], AF.Relu,
                                     bias=bias1[:, 1:2], scale=1.0 / S12)
                r1 = r_pool.tile([128, COLS], dt.bfloat16, name="r1", tag="r")
                nc.scalar.activation(r1[:], ps1[0][:], AF.Relu,
                                     bias=bias1[:, 0:1], scale=1.0 / S12)

                # -- DVE: z2(i) builds, direct fp8 --
                z2h = []
                for half in range(2):
                    z_t = z2q_pool.tile([128, 16, COLS], dt.float8e4,
                                        name=f"z2q{half}", tag="z2q")
                    nc.vector.tensor_tensor(
                        z_t[:], xrt[i][:, half * 16:(half + 1) * 16, :],
                        h2[:].unsqueeze(1).broadcast_to([128, 16, COLS]),
                        ALU.mult)
                    z2h.append(z_t)
                z2t[i] = z2h

                reduce_into(P1, i, r1)
                if i == n_tiles - 1:
                    nc.sync.dma_start(pout.ap()[0], P0[:])
                    nc.sync.dma_start(pout.ap()[1], P1[:])

        nc.sync.dma_start(pout.ap()[2], P2a[:])
        nc.sync.dma_start(pout.ap()[3], P2b[:])

    nc.compile()
    return nc


def _pair_indices():
    """Unordered-pair enumeration for the symmetric L0 contraction:
    32 diagonal pairs first, then the 496 f1<f2 pairs (total 528)."""
    ia = [f for f in range(F)]
    ib = [f for f in range(F)]
    for f1 in range(F):
        for f2 in range(f1 + 1, F):
            ia.append(f1)
            ib.append(f2)
    return np.asarray(ia, np.int64), np.asarray(ib, np.int64)


def _prep_inputs(x, w0, b0, w1, b1, w2, b2, fc_w, fc_b):
    bf16 = ml_dtypes.bfloat16
    f8 = ml_dtypes.float8_e4m3
    xb = np.asarray(x, np.float32).astype(bf16)

    ia, ib = _pair_indices()
    iap = np.zeros(768, np.int64); iap[:528] = ia
    ibp = np.zeros(768, np.int64); ibp[:528] = ib

    # L0 symmetrization (exact bf16 path): 528 pairs padded to 768
    w0f = np.asarray(w0, np.float32).reshape(O, F, F)
    w0s = np.zeros((O, 768), np.float32)
    w0s[:, :528] = w0f[:, ia, ib]
    off = ia != ib
    w0s[:, :528][:, off] += w0f[:, ib[off], ia[off]]
    # w0t: [p, (g, o)] with channel c = 128*g + p
    w0t = np.ascontiguousarray(
        w0s.T.reshape(NP0, 128, O).transpose(1, 0, 2).reshape(128, NP0 * O)
    ).astype(bf16)

    def wprep_res(w):
        # fp8 hi + unscaled lo residual; layout [p, m, hl, g, o']
        ws = np.asarray(w, np.float32).T * S12          # [cin, O]
        hi = ws.astype(f8)
        lo = (ws - hi.astype(np.float32)).astype(f8)
        out = np.empty((128, 2, 2, 32, 128), f8)
        for hl, arr in enumerate((hi, lo)):
            a = arr.reshape(32, 128, 2, 128)            # [g, p, m, o']
            out[:, :, hl] = a.transpose(1, 2, 0, 3)     # -> [p, m, g, o']
        return np.ascontiguousarray(out.reshape(128, 2 * 2 * 32 * 128))

    common = {
        "w0t": w0t, "w1t": wprep_res(w1), "w2t": wprep_res(w2),
        "b0": np.ascontiguousarray(np.asarray(b0, np.float32)),
        "b1": np.ascontiguousarray(np.asarray(b1, np.float32)),
        "b2": np.ascontiguousarray(np.asarray(b2, np.float32)),
    }
    in_maps = []
    for c in range(N_CORES):
        m = dict(common)
        xc = xb[c * BC:(c + 1) * BC]                     # [BC, F, E]
        xt = np.ascontiguousarray(
            xc.reshape(NT, NB, F, E).transpose(0, 2, 1, 3).reshape(NT, F, COLS))
        m["xtile"] = xt
        # z0q[t, p, (g c)] = xt[t, iap[128g+p]] * xt[t, ibp[128g+p]]
        xf = xt.astype(np.float32)
        g = xf[:, iap, :] * xf[:, ibp, :]                # [NT, 768, COLS]
        m["z0q"] = np.ascontiguousarray(
            g.reshape(NT, NP0, 128, COLS).transpose(0, 2, 1, 3)
             .reshape(NT, 128, NP0 * COLS).astype(bf16))
        in_maps.append(m)
    return in_maps


def kernel(x, w0, b0, w1, b1, w2, b2, fc_w, fc_b, **kw):
    from concourse.bass_utils import run_bass_kernel_spmd

    if "nc" not in _CACHE:
        _CACHE["nc"] = _build()
    nc = _CACHE["nc"]
    in_maps = _prep_inputs(x, w0, b0, w1, b1, w2, b2, fc_w, fc_b)
    res = run_bass_kernel_spmd(nc, in_maps, list(range(N_CORES)))
    fcw = np.asarray(fc_w, np.float32).reshape(4, 128)
    ys = []
    for c in range(N_CORES):
        p = res.results[c]["pout"]  # [4, 128, BC]
        ys.append(np.einsum('cp,cpb->b', fcw, p.astype(np.float32)))
    out = np.concatenate(ys).reshape(B, 1).astype(np.float32)
    out = out + np.asarray(fc_b, np.float32).reshape(1, 1)
    return out


# revision 8
# speedup vs baseline: 1.1197x; 1.1197x over previous
"""CIN (xDeepFM CompressedInteractionNetwork) forward on 8 TRN2 NeuronCores.

v3: engine-balanced partial-fp8. Measured HW rates (v2 trace): DoubleRow
fp8 matmul 234ns (=157TF/s, 2x bf16), DVE TT bf16-out 0.54ns/elem,
fp8-out 1.04ns/elem, Act convert 0.85ns/elem, Pool mult 2.02ns/elem.
Converting z to fp8 costs more engine-time than it saves on the PE, so
fp8 pays only as load-balancing: the LP-optimal split keeps 3/16 of the
k-subtile quarters in bf16 and lands every engine at ~20us/tile.

  - batch 4096 -> 512/core; 64 tiles x (8 batch x 64 embed) = 512 cols.
  - L0 exact bf16: host-precomputed symmetrized pair products (528 pairs
    padded to 768 = 6x128), 6 bf16 matmuls per out-half.
  - L1/L2 k-subtiles split by f-range into four classes:
      D  direct-fp8:   DVE builds z in fp8 (1 op), fp8 DR matmuls
      A  bf16+ActConv: DVE builds bf16, ScalarE converts to fp8, DR
      P  Pool+ActConv: GpSimd builds bf16, ScalarE converts, DR
      S  stay-bf16:    DVE builds bf16, plain bf16 matmuls
    L1: D=f0:8 A=f8:16 P=f16:24 S=f24:32; L2: D=f0:8 A=f8:20 P=f20:28
    S=f28:32. fp8 weights scaled x64 (ScalarE un-scales via 1/64 in the
    bias+relu activation); the stay-class bf16 weights are scaled x64
    too so one PSUM accumulation group mixes both dtypes.
  - pipeline: PE iter i = [L0(i), L2(i-2), L1(i)]; z1(i) built iter i;
    z2(i)'s DVE parts built at iter i+1 HEAD (h2(i) is ready, so the DVE
    never stalls), Pool's z2 part at iter-i tail; all z2 converts retire
    during iter i+1, one full iteration before L2(i) consumes them.
  - pooling on DVE; final FC on host.
"""

import sys

sys.path.insert(0, "/opt/trn_rl_repo")

import numpy as np
import ml_dtypes
from contextlib import ExitStack

N_CORES = 8
B = 4096
F = 32
E = 64
BC = B // N_CORES  # 512 batch elements per core
NB = 8             # batch elements per tile
COLS = NB * E      # 512 matmul columns per tile
NT = BC // NB      # 64 tiles per core
O = 256            # conv out channels per layer
NP0 = 6            # L0 k-subtiles: 528 pairs padded to 768 = 6*128
S12 = 64.0         # fp8 weight scale for L1/L2

# class boundaries (f ranges): [D_end, A_end, P_end] with S = rest
CLS1 = (8, 16, 24)   # L1: D 8, A 8, P 8, S 8
CLS2 = (8, 20, 28)   # L2: D 8, A 12, P 8, S 4

_CACHE = {}


def _build(n_tiles=NT):
    import concourse.bass as bass  # noqa: F401
    import concourse.mybir as mybir
    import concourse.tile as tile
    from concourse import bacc

    dt = mybir.dt
    AF = mybir.ActivationFunctionType
    ALU = mybir.AluOpType
    AX = mybir.AxisListType
    DR = mybir.MatmulPerfMode.DoubleRow

    nc = bacc.Bacc("TRN2", target_bir_lowering=False, debug=False,
                   num_devices=N_CORES)

    z0q = nc.declare_dram_parameter("z0q", [n_tiles, 128, NP0 * COLS],
                                    dt.bfloat16, isOutput=False)
    xtile = nc.declare_dram_parameter("xtile", [n_tiles, F, COLS],
                                      dt.bfloat16, isOutput=False)
    w0t = nc.declare_dram_parameter("w0t", [128, NP0 * O], dt.bfloat16,
                                    isOutput=False)
    # fp8 weights cover D+A+P subtiles; bf16 weights (also x64) cover S
    w1f8 = nc.declare_dram_parameter("w1f8", [128, 2 * CLS1[2] * 128],
                                     dt.float8e4, isOutput=False)
    w1bf = nc.declare_dram_parameter("w1bf", [128, 2 * (F - CLS1[2]) * 128],
                                     dt.bfloat16, isOutput=False)
    w2f8 = nc.declare_dram_parameter("w2f8", [128, 2 * CLS2[2] * 128],
                                     dt.float8e4, isOutput=False)
    w2bf = nc.declare_dram_parameter("w2bf", [128, 2 * (F - CLS2[2]) * 128],
                                     dt.bfloat16, isOutput=False)
    b0 = nc.declare_dram_parameter("b0", [O], dt.float32, isOutput=False)
    b1 = nc.declare_dram_parameter("b1", [O], dt.float32, isOutput=False)
    b2 = nc.declare_dram_parameter("b2", [O], dt.float32, isOutput=False)
    pout = nc.declare_dram_parameter("pout", [4, 128, n_tiles * NB],
                                     dt.float32, isOutput=True)

    with ExitStack() as ctx:
        tc = ctx.enter_context(tile.TileContext(nc))
        const = ctx.enter_context(tc.tile_pool(name="const", bufs=1))

        lw0 = const.tile([128, NP0, O], dt.bfloat16)
        lw1f = const.tile([128, 2, CLS1[2], 128], dt.float8e4)
        lw1b = const.tile([128, 2, F - CLS1[2], 128], dt.bfloat16)
        lw2f = const.tile([128, 2, CLS2[2], 128], dt.float8e4)
        lw2b = const.tile([128, 2, F - CLS2[2], 128], dt.bfloat16)
        bias0 = const.tile([128, 2], dt.float32)
        bias1 = const.tile([128, 2], dt.float32)
        bias2 = const.tile([128, 2], dt.float32)

        P0 = const.tile([128, n_tiles * NB], dt.float32)
        P1 = const.tile([128, n_tiles * NB], dt.float32)
        P2a = const.tile([128, n_tiles * NB], dt.float32)
        P2b = const.tile([128, n_tiles * NB], dt.float32)

        z0_pool = ctx.enter_context(tc.tile_pool(name="z0", bufs=2))
        xr_pool = ctx.enter_context(tc.tile_pool(name="xr", bufs=2))
        z1_pool = ctx.enter_context(tc.tile_pool(name="z1", bufs=1))
        z2_pool = ctx.enter_context(tc.tile_pool(name="z2", bufs=2))
        h_pool = ctx.enter_context(tc.tile_pool(name="h", bufs=2))
        r_pool = ctx.enter_context(tc.tile_pool(name="r", bufs=2))
        psum_pool = ctx.enter_context(tc.tile_pool(name="ps", bufs=8, space="PSUM"))

        z0t = [None] * n_tiles
        xrt = [None] * n_tiles
        z1t = [None] * n_tiles   # dict cls -> tile
        z2t = [None] * n_tiles
        z2pre = [None] * n_tiles  # (z2ab, z2pb) bf16 tiles pending convert
        h2t = [None] * n_tiles

        def emit_dma(t):
            z0 = z0_pool.tile([128, NP0, COLS], dt.bfloat16)
            nc.sync.dma_start(z0[:].rearrange("p g c -> p (g c)"), z0q.ap()[t])
            z0t[t] = z0
            xr = xr_pool.tile([128, F, COLS], dt.bfloat16)
            src = xtile.ap()[t].unsqueeze(0).broadcast_to([128, F, COLS])
            nc.sync.dma_start(xr[:], src)
            xrt[t] = xr

        def reduce_into(P, t, r_t):
            nc.vector.tensor_reduce(
                P[:, t * NB:(t + 1) * NB],
                r_t[:].rearrange("p (b e) -> p b e", e=E), AX.X, ALU.add)

        def hbc(h, n):
            return h[:].unsqueeze(1).broadcast_to([128, n, COLS])

        def build(pool_, nm, tag, lo, hi, xr, h, eng, odt, bufs=None):
            ztile = pool_.tile([128, hi - lo, COLS], odt, name=nm, tag=tag,
                               **({"bufs": bufs} if bufs else {}))
            eng.tensor_tensor(ztile[:], xr[:, lo:hi, :], hbc(h, hi - lo),
                              ALU.mult)
            return ztile

        def conv(pool_, nm, tag, src, bufs=None):
            q = pool_.tile([128, src.shape[1], COLS], dt.float8e4, name=nm,
                           tag=tag, **({"bufs": bufs} if bufs else {}))
            nc.scalar.activation(q[:], src[:], AF.Copy)
            return q

        def emit_chain(ps, m, lwf, lwb, cls, z):
            """One m-chain: D (fp8 DR), S (bf16), A, P (fp8 DR)."""
            d_end, a_end, p_end = cls
            ops = []
            for g in range(d_end // 2):                      # D
                ops.append(('f8', 2 * g, z['d'], 2 * g - 0))
            for s in range(F - p_end):                       # S
                ops.append(('bf', s, z['s'], s))
            for g in range((a_end - d_end) // 2):            # A
                ops.append(('f8', d_end + 2 * g, z['a'], 2 * g))
            for g in range((p_end - a_end) // 2):            # P
                ops.append(('f8', a_end + 2 * g, z['p'], 2 * g))
            n = len(ops)
            for k, (kind, wg, ztile, zg) in enumerate(ops):
                if kind == 'f8':
                    nc.tensor.matmul(
                        ps[:], lwf[:, m, wg:wg + 2, :],
                        ztile[:, zg:zg + 2, :],
                        start=(k == 0), stop=(k == n - 1), perf_mode=DR)
                else:
                    nc.tensor.matmul(
                        ps[:], lwb[:, m, wg, :], ztile[:, zg, :],
                        start=(k == 0), stop=(k == n - 1))

        # ---- preamble ----
        emit_dma(0)
        nc.sync.dma_start(lw0[:], w0t.ap().rearrange("p (g o) -> p g o", o=O))
        nc.sync.dma_start(bias0[:], b0.ap().rearrange("(m p) -> p m", p=128))
        w1fv = w1f8.ap().rearrange("p (m x) -> p m x", m=2)
        w1bv = w1bf.ap().rearrange("p (m x) -> p m x", m=2)
        w2fv = w2f8.ap().rearrange("p (m x) -> p m x", m=2)
        w2bv = w2bf.ap().rearrange("p (m x) -> p m x", m=2)
        for m in (1, 0):
            nc.sync.dma_start(lw1f[:, m].rearrange("p g o -> p (g o)"), w1fv[:, m])
            nc.sync.dma_start(lw1b[:, m].rearrange("p g o -> p (g o)"), w1bv[:, m])
        nc.sync.dma_start(bias1[:], b1.ap().rearrange("(m p) -> p m", p=128))
        for m in (0, 1):
            nc.sync.dma_start(lw2f[:, m].rearrange("p g o -> p (g o)"), w2fv[:, m])
            nc.sync.dma_start(lw2b[:, m].rearrange("p g o -> p (g o)"), w2bv[:, m])
        nc.sync.dma_start(bias2[:], b2.ap().rearrange("(m p) -> p m", p=128))

        for i in range(n_tiles + 2):
            if i + 1 < n_tiles:
                emit_dma(i + 1)

            # -- DVE head: z2(i-1) DVE-built parts (h2(i-1) ready) --
            if 1 <= i <= n_tiles and i - 1 < n_tiles:
                c = i - 1
                xr, h2 = xrt[c], h2t[c]
                d_end, a_end, p_end = CLS2
                z2d = build(z2_pool, "z2d", "z2d", 0, d_end, xr, h2,
                            nc.vector, dt.float8e4)
                z2ab = build(z2_pool, "z2ab", "z2ab", d_end, a_end, xr, h2,
                             nc.vector, dt.bfloat16, bufs=1)
                z2s = build(z2_pool, "z2s", "z2s", p_end, F, xr, h2,
                            nc.vector, dt.bfloat16)
                z2pre[c] = (z2ab, z2pre[c])  # (ab, pb already set at tail)
                z2t[c] = {'d': z2d, 's': z2s}

            if i < n_tiles:
                # -- PE: L0(i) bf16 exact; m=1 first --
                ps0 = {m: psum_pool.tile([128, COLS], dt.float32,
                                         name=f"ps0{m}", tag="ps")
                       for m in (1, 0)}
                for m in (1, 0):
                    for g in range(NP0):
                        nc.tensor.matmul(
                            ps0[m][:], lw0[:, g, m * 128:(m + 1) * 128],
                            z0t[i][:, g, :], start=(g == 0), stop=(g == NP0 - 1))
                h1 = h_pool.tile([128, COLS], dt.bfloat16, name="h1", tag="h1")
                nc.scalar.activation(h1[:], ps0[1][:], AF.Relu, bias=bias0[:, 1:2])
                r0 = r_pool.tile([128, COLS], dt.bfloat16, name="r0", tag="r0", bufs=1)
                nc.scalar.activation(r0[:], ps0[0][:], AF.Relu, bias=bias0[:, 0:1])

                # -- z1(i) builds: DVE [A, D, S], Pool [P] --
                d_end, a_end, p_end = CLS1
                xr = xrt[i]
                z1ab = build(z1_pool, "z1ab", "z1ab", d_end, a_end, xr, h1,
                             nc.vector, dt.bfloat16)
                z1d = build(z1_pool, "z1d", "z1d", 0, d_end, xr, h1,
                            nc.vector, dt.float8e4)
                z1s = build(z1_pool, "z1s", "z1s", p_end, F, xr, h1,
                            nc.vector, dt.bfloat16)
                z1pb = build(z1_pool, "z1pb", "z1pb", a_end, p_end, xr, h1,
                             nc.gpsimd, dt.bfloat16)
                z1aq = conv(z1_pool, "z1aq", "z1aq", z1ab)
                z1pq = conv(z1_pool, "z1pq", "z1pq", z1pb)
                z1t[i] = {'d': z1d, 'a': z1aq, 'p': z1pq, 's': z1s}

            # -- Act: z2(i-1) A-convert (input built this iter's DVE head) --
            if 1 <= i <= n_tiles:
                c = i - 1
                z2ab, z2pb = z2pre[c]
                z2t[c]['a'] = conv(z2_pool, "z2aq", "z2aq", z2ab)

            if i >= 2 and i - 2 < n_tiles:
                # -- PE: L2(i-2) mixed chains --
                c = i - 2
                ps2 = {m: psum_pool.tile([128, COLS], dt.float32,
                                         name=f"ps2{m}", tag="ps")
                       for m in (0, 1)}
                for m in (0, 1):
                    emit_chain(ps2[m], m, lw2f, lw2b, CLS2, z2t[c])
                r2a = r_pool.tile([128, COLS], dt.bfloat16, name="r2a", tag="r2a", bufs=1)
                nc.scalar.activation(r2a[:], ps2[0][:], AF.Relu,
                                     bias=bias2[:, 0:1], scale=1.0 / S12)
                r2b = r_pool.tile([128, COLS], dt.bfloat16, name="r2b", tag="r2b", bufs=1)
                nc.scalar.activation(r2b[:], ps2[1][:], AF.Relu,
                                     bias=bias2[:, 1:2], scale=1.0 / S12)

            if i < n_tiles:
                # -- PE: L1(i) mixed chains; m=1 first --
                ps1 = {m: psum_pool.tile([128, COLS], dt.float32,
                                         name=f"ps1{m}", tag="ps")
                       for m in (1, 0)}
                for m in (1, 0):
                    emit_chain(ps1[m], m, lw1f, lw1b, CLS1, z1t[i])
                h2 = h_pool.tile([128, COLS], dt.bfloat16, name="h2", tag="h2")
                nc.scalar.activation(h2[:], ps1[1][:], AF.Relu,
                                     bias=bias1[:, 1:2], scale=1.0 / S12)
                r1 = r_pool.tile([128, COLS], dt.bfloat16, name="r1", tag="r1")
                nc.scalar.activation(r1[:], ps1[0][:], AF.Relu,
                                     bias=bias1[:, 0:1], scale=1.0 / S12)
                h2t[i] = h2
                r1t = r1

            # -- DVE reduces (lagged off the critical path) --
            if i < n_tiles:
                reduce_into(P0, i, r0)
            if i >= 2 and i - 2 < n_tiles:
                reduce_into(P2a, i - 2, r2a)
                reduce_into(P2b, i - 2, r2b)
            if 1 <= i <= n_tiles:
                reduce_into(P1, i - 1, r1prev)
            if i < n_tiles:
                r1prev = r1t

            # -- Act tail: z2(i-1) P-convert (needed at iter i+1) --
            if 1 <= i <= n_tiles:
                c = i - 1
                z2t[c]['p'] = conv(z2_pool, "z2pq", "z2pq", z2pre[c][1])

            # -- Pool tail: z2(i) Pool-built part (after h2(i)) --
            if i < n_tiles:
                d_end, a_end, p_end = CLS2
                z2pb = build(z2_pool, "z2pb", "z2pb", a_end, p_end, xrt[i],
                             h2t[i], nc.gpsimd, dt.bfloat16, bufs=1)
                z2pre[i] = z2pb  # completed to (ab, pb) at iter i+1 head

            if i == n_tiles - 1:
                nc.sync.dma_start(pout.ap()[0], P0[:])
            if i == n_tiles:
                nc.sync.dma_start(pout.ap()[1], P1[:])

        nc.sync.dma_start(pout.ap()[2], P2a[:])
        nc.sync.dma_start(pout.ap()[3], P2b[:])

    nc.compile()
    return nc


def _pair_indices():
    ia = [f for f in range(F)]
    ib = [f for f in range(F)]
    for f1 in range(F):
        for f2 in range(f1 + 1, F):
            ia.append(f1)
            ib.append(f2)
    return np.asarray(ia, np.int64), np.asarray(ib, np.int64)


def _prep_inputs(x, w0, b0, w1, b1, w2, b2, fc_w, fc_b):
    bf16 = ml_dtypes.bfloat16
    f8 = ml_dtypes.float8_e4m3
    xb = np.asarray(x, np.float32).astype(bf16)

    ia, ib = _pair_indices()
    iap = np.zeros(768, np.int64); iap[:528] = ia
    ibp = np.zeros(768, np.int64); ibp[:528] = ib

    w0f = np.asarray(w0, np.float32).reshape(O, F, F)
    w0s = np.zeros((O, 768), np.float32)
    w0s[:, :528] = w0f[:, ia, ib]
    off = ia != ib
    w0s[:, :528][:, off] += w0f[:, ib[off], ia[off]]
    w0t = np.ascontiguousarray(
        w0s.T.reshape(NP0, 128, O).transpose(1, 0, 2).reshape(128, NP0 * O)
    ).astype(bf16)

    def wprep_mixed(w, cls):
        # [cin, O] -> [g(f), p, m, o']; fp8 part f<cls[2] (x64), bf16 rest (x64)
        ws = np.asarray(w, np.float32).T.reshape(F, 128, 2, 128) * S12
        wf = ws[:cls[2]].astype(f8).transpose(1, 2, 0, 3)
        wb = ws[cls[2]:].astype(bf16).transpose(1, 2, 0, 3)
        return (np.ascontiguousarray(wf.reshape(128, -1)),
                np.ascontiguousarray(wb.reshape(128, -1)))

    w1f, w1b_ = wprep_mixed(w1, CLS1)
    w2f, w2b_ = wprep_mixed(w2, CLS2)
    common = {
        "w0t": w0t, "w1f8": w1f, "w1bf": w1b_, "w2f8": w2f, "w2bf": w2b_,
        "b0": np.ascontiguousarray(np.asarray(b0, np.float32)),
        "b1": np.ascontiguousarray(np.asarray(b1, np.float32)),
        "b2": np.ascontiguousarray(np.asarray(b2, np.float32)),
    }
    in_maps = []
    for c in range(N_CORES):
        m = dict(common)
        xc = xb[c * BC:(c + 1) * BC]
        xt = np.ascontiguousarray(
            xc.reshape(NT, NB, F, E).transpose(0, 2, 1, 3).reshape(NT, F, COLS))
        m["xtile"] = xt
        xf = xt.astype(np.float32)
        g = xf[:, iap, :] * xf[:, ibp, :]
        m["z0q"] = np.ascontiguousarray(
            g.reshape(NT, NP0, 128, COLS).transpose(0, 2, 1, 3)
             .reshape(NT, 128, NP0 * COLS).astype(bf16))
        in_maps.append(m)
    return in_maps


def kernel(x, w0, b0, w1, b1, w2, b2, fc_w, fc_b, **kw):
    from concourse.bass_utils import run_bass_kernel_spmd

    if "nc" not in _CACHE:
        _CACHE["nc"] = _build()
    nc = _CACHE["nc"]
    in_maps = _prep_inputs(x, w0, b0, w1, b1, w2, b2, fc_w, fc_b)
    res = run_bass_kernel_spmd(nc, in_maps, list(range(N_CORES)))
    fcw = np.asarray(fc_w, np.float32).reshape(4, 128)
    ys = []
    for c in range(N_CORES):
        p = res.results[c]["pout"]  # [4, 128, BC]
        ys.append(np.einsum('cp,cpb->b', fcw, p.astype(np.float32)))
    out = np.concatenate(ys).reshape(B, 1).astype(np.float32)
    out = out + np.asarray(fc_b, np.float32).reshape(1, 1)
    return out


# revision 9
# speedup vs baseline: 1.3739x; 1.2271x over previous
"""CIN (xDeepFM CompressedInteractionNetwork) forward on 8 TRN2 NeuronCores.

v4: engine-balanced partial-fp8 at NB=4 (256-col tiles) so every rotating
tile is double-buffered in SBUF — v3's tight rings WAR-coupled the engines
and inflated the loop ~2x over its busy time.

Measured HW rates (v2/v3 traces): DR fp8 matmul ~0.46ns/col (157TF/s, 2x
bf16), DVE TT bf16-out 0.556ns/elem, fp8-out 1.08, Act convert 0.89,
GpSimd TT w/broadcast 2.85. Converting z to fp8 costs more engine-time
than it saves on the PE, so fp8 is load-balancing, not work reduction.

  - batch 4096 -> 512/core; 128 tiles x (4 batch x 64 embed) = 256 cols.
  - L0 exact bf16: host-precomputed symmetrized pair products (528 pairs
    padded to 768 = 6x128), 6 bf16 matmuls per out-half.
  - L1 k-subtiles: A = f0:12 (DVE bf16 build, per-quarter ScalarE converts
    to fp8, DR matmuls), S = f12:32 (stay bf16, plain matmuls). The z1
    critical chain h1 -> build-q -> convert-q pipelines at quarter
    granularity and is ready before the PE reaches L1 in the same
    iteration.
  - L2 k-subtiles (2 iterations of slack): D = f0:12 (DVE direct-fp8
    build), A = f12:20 (DVE bf16 + convert), P = f20:32 (GpSimd bf16 +
    convert; the slow Pool engine only ever touches slack work). z2(i)'s
    DVE parts build at iter i+1's head off h2(i); the Pool part launches
    at iter i's tail.
  - fp8 weights x64 (un-scaled by 1/64 inside the bias+relu activation);
    stay-class bf16 weights also x64 so one PSUM group mixes dtypes.
  - PE iteration i = [L0(i), L2(i-2), L1(i)]; pooling reduces on DVE;
    final FC on host.
"""

import sys

sys.path.insert(0, "/opt/trn_rl_repo")

import numpy as np
import ml_dtypes
from contextlib import ExitStack

N_CORES = 8
B = 4096
F = 32
E = 64
BC = B // N_CORES  # 512 batch elements per core
NB = 4             # batch elements per tile
COLS = NB * E      # 256 matmul columns per tile
NT = BC // NB      # 128 tiles per core
O = 256            # conv out channels per layer
NP0 = 6            # L0 k-subtiles: 528 pairs padded to 768 = 6*128
S12 = 64.0         # fp8 weight scale for L1/L2

A1 = 12            # L1: A = f0:A1 (fp8 via convert), S = f|A1:32 (bf16)
D2, A2 = 12, 20    # L2: D = f0:12 direct-fp8, A = f12:20, P = f20:32

_CACHE = {}


def _build(n_tiles=NT):
    import concourse.bass as bass  # noqa: F401
    import concourse.mybir as mybir
    import concourse.tile as tile
    from concourse import bacc

    dt = mybir.dt
    AF = mybir.ActivationFunctionType
    ALU = mybir.AluOpType
    AX = mybir.AxisListType
    DR = mybir.MatmulPerfMode.DoubleRow

    nc = bacc.Bacc("TRN2", target_bir_lowering=False, debug=False,
                   num_devices=N_CORES)

    z0q = nc.declare_dram_parameter("z0q", [n_tiles, 128, NP0 * COLS],
                                    dt.bfloat16, isOutput=False)
    xtile = nc.declare_dram_parameter("xtile", [n_tiles, F, COLS],
                                      dt.bfloat16, isOutput=False)
    w0t = nc.declare_dram_parameter("w0t", [128, NP0 * O], dt.bfloat16,
                                    isOutput=False)
    w1f8 = nc.declare_dram_parameter("w1f8", [128, 2 * A1 * 128],
                                     dt.float8e4, isOutput=False)
    w1bf = nc.declare_dram_parameter("w1bf", [128, 2 * (F - A1) * 128],
                                     dt.bfloat16, isOutput=False)
    w2f8 = nc.declare_dram_parameter("w2f8", [128, 2 * F * 128],
                                     dt.float8e4, isOutput=False)
    b0 = nc.declare_dram_parameter("b0", [O], dt.float32, isOutput=False)
    b1 = nc.declare_dram_parameter("b1", [O], dt.float32, isOutput=False)
    b2 = nc.declare_dram_parameter("b2", [O], dt.float32, isOutput=False)
    pout = nc.declare_dram_parameter("pout", [4, 128, n_tiles * NB],
                                     dt.float32, isOutput=True)

    with ExitStack() as ctx:
        tc = ctx.enter_context(tile.TileContext(nc))
        const = ctx.enter_context(tc.tile_pool(name="const", bufs=1))

        lw0 = const.tile([128, NP0, O], dt.bfloat16)
        lw1f = const.tile([128, 2, A1, 128], dt.float8e4)
        lw1b = const.tile([128, 2, F - A1, 128], dt.bfloat16)
        lw2f = const.tile([128, 2, F, 128], dt.float8e4)
        bias0 = const.tile([128, 2], dt.float32)
        bias1 = const.tile([128, 2], dt.float32)
        bias2 = const.tile([128, 2], dt.float32)

        P0 = const.tile([128, n_tiles * NB], dt.float32)
        P1 = const.tile([128, n_tiles * NB], dt.float32)
        P2a = const.tile([128, n_tiles * NB], dt.float32)
        P2b = const.tile([128, n_tiles * NB], dt.float32)

        z0_pool = ctx.enter_context(tc.tile_pool(name="z0", bufs=2))
        xr_pool = ctx.enter_context(tc.tile_pool(name="xr", bufs=3))
        z1_pool = ctx.enter_context(tc.tile_pool(name="z1", bufs=2))
        z2_pool = ctx.enter_context(tc.tile_pool(name="z2", bufs=2))
        h_pool = ctx.enter_context(tc.tile_pool(name="h", bufs=2))
        r_pool = ctx.enter_context(tc.tile_pool(name="r", bufs=2))
        psum_pool = ctx.enter_context(tc.tile_pool(name="ps", bufs=8, space="PSUM"))

        z0t = [None] * n_tiles
        xrt = [None] * n_tiles
        z1t = [None] * n_tiles
        z2t = [None] * n_tiles   # dict: d, aq, pq (set across iters)
        z2pb_t = [None] * n_tiles
        h2t = [None] * n_tiles
        r1t = [None] * n_tiles

        def emit_dma(t):
            z0 = z0_pool.tile([128, NP0, COLS], dt.bfloat16)
            nc.sync.dma_start(z0[:].rearrange("p g c -> p (g c)"), z0q.ap()[t])
            z0t[t] = z0
            xr = xr_pool.tile([128, F, COLS], dt.bfloat16)
            src = xtile.ap()[t].unsqueeze(0).broadcast_to([128, F, COLS])
            nc.sync.dma_start(xr[:], src)
            xrt[t] = xr

        def reduce_into(P, t, r_t):
            nc.vector.tensor_reduce(
                P[:, t * NB:(t + 1) * NB],
                r_t[:].rearrange("p (b e) -> p b e", e=E), AX.X, ALU.add)

        def hbc(h, n):
            return h[:].unsqueeze(1).broadcast_to([128, n, COLS])

        # ---- preamble ----
        emit_dma(0)
        nc.sync.dma_start(lw0[:], w0t.ap().rearrange("p (g o) -> p g o", o=O))
        nc.sync.dma_start(bias0[:], b0.ap().rearrange("(m p) -> p m", p=128))
        w1fv = w1f8.ap().rearrange("p (m x) -> p m x", m=2)
        w1bv = w1bf.ap().rearrange("p (m x) -> p m x", m=2)
        w2fv = w2f8.ap().rearrange("p (m x) -> p m x", m=2)
        for m in (1, 0):
            nc.sync.dma_start(lw1f[:, m].rearrange("p g o -> p (g o)"), w1fv[:, m])
            nc.sync.dma_start(lw1b[:, m].rearrange("p g o -> p (g o)"), w1bv[:, m])
        nc.sync.dma_start(bias1[:], b1.ap().rearrange("(m p) -> p m", p=128))
        for m in (0, 1):
            nc.sync.dma_start(lw2f[:, m].rearrange("p g o -> p (g o)"), w2fv[:, m])
        nc.sync.dma_start(bias2[:], b2.ap().rearrange("(m p) -> p m", p=128))

        for i in range(n_tiles + 2):
            if i + 1 < n_tiles:
                emit_dma(i + 1)

            # -- DVE: one z2(i-1) quarter to fill the h1-wait bubble --
            c = i - 1
            if 0 <= c < n_tiles:
                z2d = z2_pool.tile([128, D2, COLS], dt.float8e4,
                                   name="z2d", tag="z2d")
                nc.vector.tensor_tensor(
                    z2d[:, 0:4, :], xrt[c][:, 0:4, :], hbc(h2t[c], 4), ALU.mult)

            if i < n_tiles:
                # -- PE: L0(i) bf16 exact; m=1 first --
                ps0 = {m: psum_pool.tile([128, COLS], dt.float32,
                                         name=f"ps0{m}", tag="ps")
                       for m in (1, 0)}
                for m in (1, 0):
                    for g in range(NP0):
                        nc.tensor.matmul(
                            ps0[m][:], lw0[:, g, m * 128:(m + 1) * 128],
                            z0t[i][:, g, :], start=(g == 0), stop=(g == NP0 - 1))
                h1 = h_pool.tile([128, COLS], dt.bfloat16, name="h1", tag="h1")
                nc.scalar.activation(h1[:], ps0[1][:], AF.Relu, bias=bias0[:, 1:2])
                r0 = r_pool.tile([128, COLS], dt.bfloat16, name="r0", tag="r0")
                nc.scalar.activation(r0[:], ps0[0][:], AF.Relu, bias=bias0[:, 0:1])

                # -- DVE: z1(i) A-quarters + S halves; Act converts per quarter --
                xr = xrt[i]
                z1ab = z1_pool.tile([128, A1, COLS], dt.bfloat16,
                                    name="z1ab", tag="z1ab")
                z1aq = z1_pool.tile([128, A1, COLS], dt.float8e4,
                                    name="z1aq", tag="z1aq")
                for q in range(A1 // 4):
                    sl = slice(4 * q, 4 * q + 4)
                    nc.vector.tensor_tensor(
                        z1ab[:, sl, :], xr[:, sl, :], hbc(h1, 4), ALU.mult)
                    nc.scalar.activation(z1aq[:, sl, :], z1ab[:, sl, :], AF.Copy)
                z1s = z1_pool.tile([128, F - A1, COLS], dt.bfloat16,
                                   name="z1s", tag="z1s")
                nc.vector.tensor_tensor(
                    z1s[:, 0:8, :], xr[:, A1:A1 + 8, :], hbc(h1, 8), ALU.mult)
                nc.vector.tensor_tensor(
                    z1s[:, 8:, :], xr[:, A1 + 8:, :], hbc(h1, F - A1 - 8),
                    ALU.mult)
                z1t[i] = (z1aq, z1s)

            # -- DVE: rest of z2(i-1); Act converts --
            if 0 <= c < n_tiles:
                for q in range(1, D2 // 4):
                    sl = slice(4 * q, 4 * q + 4)
                    nc.vector.tensor_tensor(
                        z2d[:, sl, :], xrt[c][:, sl, :], hbc(h2t[c], 4),
                        ALU.mult)
                z2ab = z2_pool.tile([128, A2 - D2, COLS], dt.bfloat16,
                                    name="z2ab", tag="z2ab")
                nc.vector.tensor_tensor(
                    z2ab[:], xrt[c][:, D2:A2, :], hbc(h2t[c], A2 - D2),
                    ALU.mult)
                z2aq = z2_pool.tile([128, A2 - D2, COLS], dt.float8e4,
                                    name="z2aq", tag="z2aq")
                nc.scalar.activation(z2aq[:], z2ab[:], AF.Copy)
                z2pq = z2_pool.tile([128, F - A2, COLS], dt.float8e4,
                                    name="z2pq", tag="z2pq")
                nc.scalar.activation(z2pq[:], z2pb_t[c][:], AF.Copy)
                z2t[c] = {'d': z2d, 'a': z2aq, 'p': z2pq}

            if i >= 2 and i - 2 < n_tiles:
                # -- PE: L2(i-2): D, A, P all fp8 DR --
                cc = i - 2
                z2 = z2t[cc]
                ps2 = {m: psum_pool.tile([128, COLS], dt.float32,
                                         name=f"ps2{m}", tag="ps")
                       for m in (0, 1)}
                for m in (0, 1):
                    k, n = 0, F // 2
                    for g in range(D2 // 2):
                        nc.tensor.matmul(
                            ps2[m][:], lw2f[:, m, 2 * g:2 * g + 2, :],
                            z2['d'][:, 2 * g:2 * g + 2, :],
                            start=(k == 0), stop=(k == n - 1), perf_mode=DR)
                        k += 1
                    for g in range((A2 - D2) // 2):
                        nc.tensor.matmul(
                            ps2[m][:], lw2f[:, m, D2 + 2 * g:D2 + 2 * g + 2, :],
                            z2['a'][:, 2 * g:2 * g + 2, :],
                            start=False, stop=(k == n - 1), perf_mode=DR)
                        k += 1
                    for g in range((F - A2) // 2):
                        nc.tensor.matmul(
                            ps2[m][:], lw2f[:, m, A2 + 2 * g:A2 + 2 * g + 2, :],
                            z2['p'][:, 2 * g:2 * g + 2, :],
                            start=False, stop=(k == n - 1), perf_mode=DR)
                        k += 1
                r2a = r_pool.tile([128, COLS], dt.bfloat16, name="r2a",
                                  tag="r2a")
                nc.scalar.activation(r2a[:], ps2[0][:], AF.Relu,
                                     bias=bias2[:, 0:1], scale=1.0 / S12)
                r2b = r_pool.tile([128, COLS], dt.bfloat16, name="r2b",
                                  tag="r2b")
                nc.scalar.activation(r2b[:], ps2[1][:], AF.Relu,
                                     bias=bias2[:, 1:2], scale=1.0 / S12)

            if i < n_tiles:
                # -- PE: L1(i): A (fp8 DR) then S (bf16); m=1 first --
                z1aq, z1s = z1t[i]
                nmm = A1 // 2 + (F - A1)
                ps1 = {m: psum_pool.tile([128, COLS], dt.float32,
                                         name=f"ps1{m}", tag="ps")
                       for m in (1, 0)}
                for m in (1, 0):
                    k = 0
                    for g in range(A1 // 2):
                        nc.tensor.matmul(
                            ps1[m][:], lw1f[:, m, 2 * g:2 * g + 2, :],
                            z1aq[:, 2 * g:2 * g + 2, :],
                            start=(k == 0), stop=(k == nmm - 1), perf_mode=DR)
                        k += 1
                    for s in range(F - A1):
                        nc.tensor.matmul(
                            ps1[m][:], lw1b[:, m, s, :], z1s[:, s, :],
                            start=False, stop=(k == nmm - 1))
                        k += 1
                h2 = h_pool.tile([128, COLS], dt.bfloat16, name="h2", tag="h2")
                nc.scalar.activation(h2[:], ps1[1][:], AF.Relu,
                                     bias=bias1[:, 1:2], scale=1.0 / S12)
                r1 = r_pool.tile([128, COLS], dt.bfloat16, name="r1", tag="r1")
                nc.scalar.activation(r1[:], ps1[0][:], AF.Relu,
                                     bias=bias1[:, 0:1], scale=1.0 / S12)
                h2t[i] = h2
                r1t[i] = r1

            # -- DVE reduces (off the critical path) --
            if i < n_tiles:
                reduce_into(P0, i, r0)
            if i >= 2 and i - 2 < n_tiles:
                reduce_into(P2a, i - 2, r2a)
                reduce_into(P2b, i - 2, r2b)
            if 1 <= i <= n_tiles:
                reduce_into(P1, i - 1, r1t[i - 1])

            # -- Pool tail: z2(i) P-part bf16 (crosses into iter i+1) --
            if i < n_tiles:
                z2pb = z2_pool.tile([128, F - A2, COLS], dt.bfloat16,
                                    name="z2pb", tag="z2pb")
                nc.gpsimd.tensor_tensor(
                    z2pb[:], xrt[i][:, A2:F, :], hbc(h2t[i], F - A2), ALU.mult)
                z2pb_t[i] = z2pb

            if i == n_tiles - 1:
                nc.sync.dma_start(pout.ap()[0], P0[:])
            if i == n_tiles:
                nc.sync.dma_start(pout.ap()[1], P1[:])

        nc.sync.dma_start(pout.ap()[2], P2a[:])
        nc.sync.dma_start(pout.ap()[3], P2b[:])

    nc.compile()
    return nc


def _pair_indices():
    ia = [f for f in range(F)]
    ib = [f for f in range(F)]
    for f1 in range(F):
        for f2 in range(f1 + 1, F):
            ia.append(f1)
            ib.append(f2)
    return np.asarray(ia, np.int64), np.asarray(ib, np.int64)


def _prep_inputs(x, w0, b0, w1, b1, w2, b2, fc_w, fc_b):
    bf16 = ml_dtypes.bfloat16
    f8 = ml_dtypes.float8_e4m3
    xb = np.asarray(x, np.float32).astype(bf16)

    ia, ib = _pair_indices()
    iap = np.zeros(768, np.int64); iap[:528] = ia
    ibp = np.zeros(768, np.int64); ibp[:528] = ib

    w0f = np.asarray(w0, np.float32).reshape(O, F, F)
    w0s = np.zeros((O, 768), np.float32)
    w0s[:, :528] = w0f[:, ia, ib]
    off = ia != ib
    w0s[:, :528][:, off] += w0f[:, ib[off], ia[off]]
    w0t = np.ascontiguousarray(
        w0s.T.reshape(NP0, 128, O).transpose(1, 0, 2).reshape(128, NP0 * O)
    ).astype(bf16)

    def wsplit(w, nf8):
        # [cin, O] -> [g(f), p, m, o']; fp8 part f<nf8 (x64), bf16 rest (x64)
        ws = np.asarray(w, np.float32).T.reshape(F, 128, 2, 128) * S12
        wf = ws[:nf8].astype(f8).transpose(1, 2, 0, 3)
        out = [np.ascontiguousarray(wf.reshape(128, -1))]
        if nf8 < F:
            wb = ws[nf8:].astype(bf16).transpose(1, 2, 0, 3)
            out.append(np.ascontiguousarray(wb.reshape(128, -1)))
        return out

    w1f, w1b_ = wsplit(w1, A1)
    (w2f,) = wsplit(w2, F)
    common = {
        "w0t": w0t, "w1f8": w1f, "w1bf": w1b_, "w2f8": w2f,
        "b0": np.ascontiguousarray(np.asarray(b0, np.float32)),
        "b1": np.ascontiguousarray(np.asarray(b1, np.float32)),
        "b2": np.ascontiguousarray(np.asarray(b2, np.float32)),
    }
    in_maps = []
    for c in range(N_CORES):
        m = dict(common)
        xc = xb[c * BC:(c + 1) * BC]
        xt = np.ascontiguousarray(
            xc.reshape(NT, NB, F, E).transpose(0, 2, 1, 3).reshape(NT, F, COLS))
        m["xtile"] = xt
        xf = xt.astype(np.float32)
        g = xf[:, iap, :] * xf[:, ibp, :]
        m["z0q"] = np.ascontiguousarray(
            g.reshape(NT, NP0, 128, COLS).transpose(0, 2, 1, 3)
             .reshape(NT, 128, NP0 * COLS).astype(bf16))
        in_maps.append(m)
    return in_maps


def kernel(x, w0, b0, w1, b1, w2, b2, fc_w, fc_b, **kw):
    from concourse.bass_utils import run_bass_kernel_spmd

    if "nc" not in _CACHE:
        _CACHE["nc"] = _build()
    nc = _CACHE["nc"]
    in_maps = _prep_inputs(x, w0, b0, w1, b1, w2, b2, fc_w, fc_b)
    res = run_bass_kernel_spmd(nc, in_maps, list(range(N_CORES)))
    fcw = np.asarray(fc_w, np.float32).reshape(4, 128)
    ys = []
    for c in range(N_CORES):
        p = res.results[c]["pout"]  # [4, 128, BC]
        ys.append(np.einsum('cp,cpb->b', fcw, p.astype(np.float32)))
    out = np.concatenate(ys).reshape(B, 1).astype(np.float32)
    out = out + np.asarray(fc_b, np.float32).reshape(1, 1)
    return out
